# revision 11
# baseline (speedup 1.0000x reference)
"""ECE loss kernel for Trainium2 (Bass/Tile), data-parallel over 8 NeuronCores.

Math (per sample row of logits[N, C]):
  conf = max softmax(x) = exp(max(x)) / sum(exp(x))
  acc  = (argmax(x) == label)
  ece  = sum_b |conf_sum[b] - acc_sum[b]| / N     (15 bins + empty tail)

Device pipeline per core (125184 rows as [128 partitions x 978 samples],
tiles of 32 samples x 100 classes):
  - DMA x tiles on the two HWDGE rings (sync/scalar)
  - ACT: E = fp16(exp(x))  (fp16 keeps the fold tree in the DVE 2x mode)
  - the per-sample max/sum reductions run as fold trees instead of full
    1-elem/cycle InstTensorReduce passes (which have no fast mode):
      DVE:  m1/s1 = 100 -> 50 pairwise max/add  (fp16 packed -> 2x mode)
      DVE:  m2/s2 = 50 -> 25 pairwise max/add   (2x mode)
      DVE:  em/s  = 25-wide reduce_max / reduce_sum  (f32 sum out)
    (the real ISA rejects TensorTensor on Pool, so the whole tree is DVE)
  - em (fp16 exact), s (f32), and gb = fp16(exp(x[label])) stream back to
    DRAM in slabs overlapped with the main loop (~8B/sample, ~2% of the
    input traffic).
Host: gathers g = x[i,label_i] (1% of bytes), computes conf = em/s and
acc = (gb == em) (both sides ACT-exp'd on device, so the compare is
table-consistent), bins, and finishes the tiny ECE reduction in float64.
"""

import os

import numpy as np

import concourse.bass as bass
import concourse.mybir as mybir
import concourse.tile as tile
from concourse.bass_utils import run_bass_kernel_spmd

F32 = mybir.dt.float32
F16 = mybir.dt.float16
ALU = mybir.AluOpType
AX = mybir.AxisListType
ACTF = mybir.ActivationFunctionType

N = 1_000_000
C = 100
NCORES = 8
P = 128
SPP = 978                   # samples per partition (padded)
ROWS = P * SPP              # 125184 rows per core
NTOT = NCORES * ROWS        # 1001472
PAD = NTOT - N              # 1472 zero pad rows on the last core
K = 64                      # samples per tile
SIZES = [16, 16, 32] + [K] * 14 + [18]   # sum = 978; small leading tiles so
                                         # the first reduces start sooner
SLABS = [256, 512, 768, 896, 978]    # em/s output DMA slab boundaries

LAST_RESULTS = None


def _build():
    nc = bass.Bass(trn_type="TRN2")
    x = nc.dram_tensor("x", [P, SPP * C], F32, kind="ExternalInput")
    g = nc.dram_tensor("g", [P, SPP], F32, kind="ExternalInput")
    em_d = nc.dram_tensor("em", [P, SPP], F16, kind="ExternalOutput")
    s_d = nc.dram_tensor("s", [P, SPP], F32, kind="ExternalOutput")
    gb_d = nc.dram_tensor("gb", [P, SPP], F16, kind="ExternalOutput")

    with tile.TileContext(nc) as tc:
        with (
            tc.tile_pool(name="xin", bufs=4) as xin,
            tc.tile_pool(name="ebp", bufs=3) as ebp,
            tc.tile_pool(name="f1p", bufs=2) as f1p,
            tc.tile_pool(name="f2p", bufs=2) as f2p,
            tc.tile_pool(name="persist", bufs=1) as persist,
        ):
            g_sb = persist.tile([P, SPP], F32)
            g_bf = persist.tile([P, SPP], F16)
            em = persist.tile([P, SPP], F16)
            s_t = persist.tile([P, SPP], F32)

            # all x loads on the SP HWDGE ring (16 queues saturate HBM; SWDGE
            # measured ~1.5x slower): SP is idle, so issue cadence never
            # couples to ACT/DVE stalls
            dma_engines = [nc.sync, nc.sync]
            off = 0
            si = 0
            with nc.allow_low_precision(reason="fp16 fold tree, validated"):
                for t, k in enumerate(SIZES):
                    kc = k * C
                    xt = xin.tile([P, K * C], F32, tag="xt")
                    dma_engines[t % 2].dma_start(
                        out=xt[:, :kc], in_=x[:, off * C : (off + k) * C]
                    )
                    if t == 0:
                        # g rides the fast sync ring right behind tile 0
                        # (SWDGE needs ~19us for these 0.5MB and would stall
                        # ACT's in-order exp(g))
                        nc.sync.dma_start(out=g_sb[:, :], in_=g[:, :])
                    eb = ebp.tile([P, K * C], F16, tag="eb")
                    nc.scalar.activation(eb[:, :kc], xt[:, :kc], ACTF.Exp)
                    ebv = eb[:, :kc].rearrange("p (k c) -> p k c", c=C)
                    m1 = f1p.tile([P, K * 50], F16, tag="m1")
                    s1 = f1p.tile([P, K * 50], F16, tag="s1")
                    m1v = m1[:, : k * 50].rearrange("p (k c) -> p k c", c=50)
                    s1v = s1[:, : k * 50].rearrange("p (k c) -> p k c", c=50)
                    nc.vector.tensor_tensor(
                        m1v, ebv[:, :, 0:50], ebv[:, :, 50:100], op=ALU.max
                    )
                    nc.vector.tensor_tensor(
                        s1v, ebv[:, :, 0:50], ebv[:, :, 50:100], op=ALU.add
                    )
                    m2 = f2p.tile([P, K * 25], F16, tag="m2")
                    s2 = f2p.tile([P, K * 25], F16, tag="s2")
                    m2v = m2[:, : k * 25].rearrange("p (k c) -> p k c", c=25)
                    s2v = s2[:, : k * 25].rearrange("p (k c) -> p k c", c=25)
                    nc.vector.tensor_tensor(
                        m2v, m1v[:, :, 0:25], m1v[:, :, 25:50], op=ALU.max
                    )
                    nc.vector.tensor_tensor(
                        s2v, s1v[:, :, 0:25], s1v[:, :, 25:50], op=ALU.add
                    )
                    nc.vector.reduce_max(
                        out=em[:, off : off + k], in_=m2v, axis=AX.X
                    )
                    nc.vector.reduce_sum(
                        out=s_t[:, off : off + k], in_=s2v, axis=AX.X
                    )
                    off += k
                    if t == 6:
                        # exp(g) is tile-independent: run it once g has
                        # surely landed (never stall ACT's in-order stream),
                        # then stream it out on the idle SWDGE ring
                        nc.scalar.activation(g_bf[:, :], g_sb[:, :], ACTF.Exp)
                        nc.gpsimd.dma_start(out=gb_d[:, :], in_=g_bf[:, :])
                    if si < len(SLABS) and off == SLABS[si]:
                        lo = SLABS[si - 1] if si else 0
                        # mid-stream outputs ride the idle SWDGE ring so they
                        # never head-of-line block the x loads on the sync
                        # ring; the final slab takes the fast sync ring (the
                        # x stream is already drained by then)
                        ring = nc.sync if si == len(SLABS) - 1 else nc.gpsimd
                        ring.dma_start(out=em_d[:, lo:off], in_=em[:, lo:off])
                        ring.dma_start(out=s_d[:, lo:off], in_=s_t[:, lo:off])
                        si += 1
            assert si == len(SLABS) and off == SPP

    # ---- sync-command budget fixes (instructions carry <= 2 sync commands:
    # completion update + at most one wait).  Drop waits provably covered by
    # earlier waits on the same engine, then split any remaining multi-wait
    # instruction across preceding same-engine drains.
    import re as _re

    def _tick_sem(name):
        return bool(_re.match(
            r"^(Activation|DVE|PE|Pool|SP|DMAHW\d+|DMASW\d+)_\d+$", name
        ))

    seen_waits = {}
    for bb in nc.m.functions[0].blocks:
        for ins in bb.instructions:
            si = ins.sync_info
            if si is None:
                continue
            tname = type(ins).__name__
            if tname == "InstEventSemaphore":
                continue
            eng = str(ins.engine).split(".")[-1]
            kept = list(si.on_wait)
            if tname not in ("InstDMACopy", "InstDrain") and len(kept) > 1:
                # same-engine waits are redundant (program order)
                kept = [w for w in kept if not w.ant_name.startswith(f"{eng}_")]
            kept2 = []
            for w in kept:
                if not _tick_sem(w.ant_name):
                    kept2.append(w)
                elif seen_waits.get((eng, w.ant_name), -1) < w.wait_value:
                    kept2.append(w)
            kept = kept2
            for w in kept:
                if not _tick_sem(w.ant_name):
                    continue
                key = (eng, w.ant_name)
                seen_waits[key] = max(seen_waits.get(key, -1), w.wait_value)
            if len(kept) != len(si.on_wait):
                si.on_wait = kept
                ins.sync_info = si

    import bass_rust as _br

    for bb in nc.m.functions[0].blocks:
        while True:
            insns = list(bb.instructions)
            target = None
            for idx, ins in enumerate(insns):
                si = ins.sync_info
                if si is None:
                    continue
                if len(si.on_wait) > 1:
                    target = (idx, ins)
                    break
            if target is None:
                break
            idx, ins = target
            si = ins.sync_info
            waits = list(si.on_wait)
            if type(ins).__name__ == "InstDrain":
                room = max(0, 1 - len(si.on_update))
            else:
                room = 1
            keep, extra = waits[len(waits) - room :], waits[: len(waits) - room]
            pos = idx
            for i, w in enumerate(extra):
                nd = mybir.InstDrain(
                    name=f"{ins.name}-presync{i}", ins=[], outs=[],
                    bass_is_fusable=False,
                )
                nd.engine = ins.engine
                nd.sync_info = _br.SyncInfo(on_wait=[w], on_update=[])
                nc.register_instruction(nd, overwrite=True)
                bb.instructions.insert(pos, nd)
                pos += 1
            si.on_wait = keep
            ins.sync_info = si
    return nc


_NC_CACHE = {}


def _get_nc():
    if "nc" not in _NC_CACHE:
        _NC_CACHE["nc"] = _build()
    return _NC_CACHE["nc"]


def kernel(logits, labels):
    global LAST_RESULTS
    logits = np.ascontiguousarray(np.asarray(logits), dtype=np.float32)
    labels_i = np.asarray(labels).astype(np.int64)
    assert logits.shape == (N, C), logits.shape

    # host-side gather of the label logit (1% of input bytes)
    gvals = logits[np.arange(N), labels_i].astype(np.float32)

    in_maps = []
    for c in range(NCORES):
        lo, hi = c * ROWS, (c + 1) * ROWS
        if hi <= N:
            xs = logits[lo:hi]
            gc = gvals[lo:hi]
        else:
            xs = np.concatenate(
                [logits[lo:], np.zeros((hi - N, C), np.float32)], axis=0
            )
            gc = np.concatenate([gvals[lo:], np.zeros(hi - N, np.float32)])
        in_maps.append(
            {
                "x": np.ascontiguousarray(xs.reshape(P, SPP * C)),
                "g": np.ascontiguousarray(gc.reshape(P, SPP)),
            }
        )

    trace = bool(int(os.environ.get("ECE_TRACE", "0")))
    res = run_bass_kernel_spmd(
        _get_nc(), in_maps, core_ids=list(range(NCORES)), trace=trace
    )
    LAST_RESULTS = res

    # host epilogue: conf/acc from the per-sample device outputs, then the
    # reference binning in float64 (pads are the positional tail; drop them)
    em = np.concatenate(
        [out["em"].reshape(-1) for out in res.results]
    )[:N]
    s = np.concatenate(
        [out["s"].reshape(-1) for out in res.results]
    )[:N]
    gb = np.concatenate(
        [out["gb"].reshape(-1) for out in res.results]
    )[:N]

    conf = em.astype(np.float64) / s.astype(np.float64)
    acc = (
        gb.view(np.uint16) == em.view(np.uint16)
    ).astype(np.float64)

    NB = 15
    bin_ids = np.clip(np.ceil(conf * NB).astype(np.int64) - 1, 0, NB)
    conf_sum = np.bincount(bin_ids, weights=conf, minlength=NB + 1)
    acc_sum = np.bincount(bin_ids, weights=acc, minlength=NB + 1)
    ece = np.abs(conf_sum - acc_sum).sum() / N
    return np.array([ece], dtype=np.float32)


# revision 12
# speedup vs baseline: 1.0815x; 1.0815x over previous
"""ECE loss kernel for Trainium2 (Bass/Tile), data-parallel over 8 NeuronCores.

Math (per sample row of logits[N, C]):
  conf = max softmax(x) = exp(max(x)) / sum(exp(x))
  acc  = (argmax(x) == label)
  ece  = sum_b |conf_sum[b] - acc_sum[b]| / N     (15 bins + empty tail)

Device pipeline per core (125184 rows as [128 partitions x 978 samples],
tiles of 64 samples x 100 classes):
  - x tiles stream on the sync HWDGE ring (16 queues, ~358 GB/s measured,
    gapless); g rides the otherwise-empty scalar HWDGE ring in parallel
  - ACT: E = fp16(exp(x))  (fp16 keeps the fold tree in the DVE 2x mode)
  - the per-sample max/sum reductions run as fold trees instead of full
    1-elem/cycle InstTensorReduce passes (which have no fast mode):
      DVE:  m1/s1 = 100 -> 50 pairwise max/add  (fp16 packed -> 2x mode)
      DVE:  m2/s2 = 50 -> 25 pairwise max/add   (2x mode)
      DVE:  em/s  = 25-wide reduce_max / reduce_sum  (f32 sum out)
    (the real ISA rejects TensorTensor on Pool, so the whole tree is DVE)
  - em (fp16 exact), s (f32), and gb = fp16(exp(x[label])) stream back to
    DRAM in double-buffered slabs overlapped with the main loop (~8B/sample,
    ~2% of the input traffic); mid-stream slabs ride the idle SWDGE ring so
    they never head-of-line block the x loads, the final slab takes the
    drained sync ring
Host: gathers g = x[i,label_i] (1% of bytes), computes conf = em/s and
acc = (gb == em) (both sides ACT-exp'd on device, so the compare is
table-consistent), bins, and finishes the tiny ECE reduction in float64.
"""

import os

import numpy as np

import concourse.bass as bass
import concourse.mybir as mybir
import concourse.tile as tile
from concourse.bass_utils import run_bass_kernel_spmd

F32 = mybir.dt.float32
F16 = mybir.dt.float16
ALU = mybir.AluOpType
AX = mybir.AxisListType
ACTF = mybir.ActivationFunctionType

N = 1_000_000
C = 100
NCORES = 8
P = 128
SPP = 978                   # samples per partition (padded)
ROWS = P * SPP              # 125184 rows per core
NTOT = NCORES * ROWS        # 1001472
PAD = NTOT - N              # 1472 zero pad rows on the last core
K = 64                      # samples per tile
SIZES = [16, 16, 32] + [K] * 14 + [18]   # sum = 978; small leading tiles so
                                         # the first reduces start sooner
SLABS = [256, 512, 768, 896, 978]    # em/s output DMA slab boundaries
SLAB_W = 256                         # slab tile width (max slab size)

LAST_RESULTS = None


def _build():
    nc = bass.Bass(trn_type="TRN2")
    x = nc.dram_tensor("x", [P, SPP * C], F32, kind="ExternalInput")
    g = nc.dram_tensor("g", [P, SPP], F32, kind="ExternalInput")
    em_d = nc.dram_tensor("em", [P, SPP], F16, kind="ExternalOutput")
    s_d = nc.dram_tensor("s", [P, SPP], F32, kind="ExternalOutput")
    gb_d = nc.dram_tensor("gb", [P, SPP], F16, kind="ExternalOutput")

    with tile.TileContext(nc) as tc:
        with (
            tc.tile_pool(name="xin", bufs=3) as xin,
            tc.tile_pool(name="ebp", bufs=4) as ebp,
            tc.tile_pool(name="f1p", bufs=2) as f1p,
            tc.tile_pool(name="f2p", bufs=2) as f2p,
            tc.tile_pool(name="outp", bufs=2) as outp,
            tc.tile_pool(name="persist", bufs=1) as persist,
        ):
            g_sb = persist.tile([P, SPP], F32)
            g_bf = persist.tile([P, SPP], F16)
            # g rides the otherwise-empty scalar HWDGE ring: it never delays
            # the x stream (sync ring) and is far faster than SWDGE
            nc.scalar.dma_start(out=g_sb[:, :], in_=g[:, :])

            off = 0
            si = 0
            slab_lo = 0
            em_sl = outp.tile([P, SLAB_W], F16, tag="em")
            s_sl = outp.tile([P, SLAB_W], F32, tag="s")
            with nc.allow_low_precision(reason="fp16 fold tree, validated"):
                for t, k in enumerate(SIZES):
                    kc = k * C
                    xt = xin.tile([P, K * C], F32, tag="xt")
                    nc.sync.dma_start(
                        out=xt[:, :kc], in_=x[:, off * C : (off + k) * C]
                    )
                    eb = ebp.tile([P, K * C], F16, tag="eb")
                    nc.scalar.activation(eb[:, :kc], xt[:, :kc], ACTF.Exp)
                    ebv = eb[:, :kc].rearrange("p (k c) -> p k c", c=C)
                    m1 = f1p.tile([P, K * 50], F16, tag="m1")
                    s1 = f1p.tile([P, K * 50], F16, tag="s1")
                    m1v = m1[:, : k * 50].rearrange("p (k c) -> p k c", c=50)
                    s1v = s1[:, : k * 50].rearrange("p (k c) -> p k c", c=50)
                    nc.vector.tensor_tensor(
                        m1v, ebv[:, :, 0:50], ebv[:, :, 50:100], op=ALU.max
                    )
                    nc.vector.tensor_tensor(
                        s1v, ebv[:, :, 0:50], ebv[:, :, 50:100], op=ALU.add
                    )
                    m2 = f2p.tile([P, K * 25], F16, tag="m2")
                    s2 = f2p.tile([P, K * 25], F16, tag="s2")
                    m2v = m2[:, : k * 25].rearrange("p (k c) -> p k c", c=25)
                    s2v = s2[:, : k * 25].rearrange("p (k c) -> p k c", c=25)
                    nc.vector.tensor_tensor(
                        m2v, m1v[:, :, 0:25], m1v[:, :, 25:50], op=ALU.max
                    )
                    nc.vector.tensor_tensor(
                        s2v, s1v[:, :, 0:25], s1v[:, :, 25:50], op=ALU.add
                    )
                    lo = off - slab_lo
                    nc.vector.reduce_max(
                        out=em_sl[:, lo : lo + k], in_=m2v, axis=AX.X
                    )
                    nc.vector.reduce_sum(
                        out=s_sl[:, lo : lo + k], in_=s2v, axis=AX.X
                    )
                    off += k
                    if t == 6:
                        # exp(g) is tile-independent: run it once g has
                        # surely landed (never stall ACT's in-order stream),
                        # then stream it out on the idle SWDGE ring
                        nc.scalar.activation(g_bf[:, :], g_sb[:, :], ACTF.Exp)
                        nc.gpsimd.dma_start(out=gb_d[:, :], in_=g_bf[:, :])
                    if si < len(SLABS) and off == SLABS[si]:
                        w = off - slab_lo
                        # mid-stream slabs ride the idle SWDGE ring so they
                        # never head-of-line block the x loads on the sync
                        # ring; the final slab takes the fast sync ring (the
                        # x stream is already drained by then).  Slab tiles
                        # are double-buffered, so later reduces never wait on
                        # an in-flight out-DMA.
                        ring = nc.sync if si == len(SLABS) - 1 else nc.gpsimd
                        ring.dma_start(
                            out=em_d[:, slab_lo:off], in_=em_sl[:, :w]
                        )
                        ring.dma_start(
                            out=s_d[:, slab_lo:off], in_=s_sl[:, :w]
                        )
                        si += 1
                        slab_lo = off
                        if si < len(SLABS):
                            em_sl = outp.tile([P, SLAB_W], F16, tag="em")
                            s_sl = outp.tile([P, SLAB_W], F32, tag="s")
            assert si == len(SLABS) and off == SPP

    # ---- sync-command budget fixes (instructions carry <= 2 sync commands:
    # completion update + at most one wait).  Drop waits provably covered by
    # earlier waits on the same engine, then split any remaining multi-wait
    # instruction across preceding same-engine drains.
    import re as _re

    def _tick_sem(name):
        return bool(_re.match(
            r"^(Activation|DVE|PE|Pool|SP|DMAHW\d+|DMASW\d+)_\d+$", name
        ))

    seen_waits = {}
    for bb in nc.m.functions[0].blocks:
        for ins in bb.instructions:
            si = ins.sync_info
            if si is None:
                continue
            tname = type(ins).__name__
            if tname == "InstEventSemaphore":
                continue
            eng = str(ins.engine).split(".")[-1]
            kept = list(si.on_wait)
            if tname not in ("InstDMACopy", "InstDrain") and len(kept) > 1:
                # same-engine waits are redundant (program order)
                kept = [w for w in kept if not w.ant_name.startswith(f"{eng}_")]
            kept2 = []
            for w in kept:
                if not _tick_sem(w.ant_name):
                    kept2.append(w)
                elif seen_waits.get((eng, w.ant_name), -1) < w.wait_value:
                    kept2.append(w)
            kept = kept2
            for w in kept:
                if not _tick_sem(w.ant_name):
                    continue
                key = (eng, w.ant_name)
                seen_waits[key] = max(seen_waits.get(key, -1), w.wait_value)
            if len(kept) != len(si.on_wait):
                si.on_wait = kept
                ins.sync_info = si

    import bass_rust as _br

    for bb in nc.m.functions[0].blocks:
        while True:
            insns = list(bb.instructions)
            target = None
            for idx, ins in enumerate(insns):
                si = ins.sync_info
                if si is None:
                    continue
                if len(si.on_wait) > 1:
                    target = (idx, ins)
                    break
            if target is None:
                break
            idx, ins = target
            si = ins.sync_info
            waits = list(si.on_wait)
            if type(ins).__name__ == "InstDrain":
                room = max(0, 1 - len(si.on_update))
            else:
                room = 1
            keep, extra = waits[len(waits) - room :], waits[: len(waits) - room]
            pos = idx
            for i, w in enumerate(extra):
                nd = mybir.InstDrain(
                    name=f"{ins.name}-presync{i}", ins=[], outs=[],
                    bass_is_fusable=False,
                )
                nd.engine = ins.engine
                nd.sync_info = _br.SyncInfo(on_wait=[w], on_update=[])
                nc.register_instruction(nd, overwrite=True)
                bb.instructions.insert(pos, nd)
                pos += 1
            si.on_wait = keep
            ins.sync_info = si
    return nc


_NC_CACHE = {}


def _get_nc():
    if "nc" not in _NC_CACHE:
        _NC_CACHE["nc"] = _build()
    return _NC_CACHE["nc"]


def kernel(logits, labels):
    global LAST_RESULTS
    logits = np.ascontiguousarray(np.asarray(logits), dtype=np.float32)
    labels_i = np.asarray(labels).astype(np.int64)
    assert logits.shape == (N, C), logits.shape

    # host-side gather of the label logit (1% of input bytes)
    gvals = logits[np.arange(N), labels_i].astype(np.float32)

    in_maps = []
    for c in range(NCORES):
        lo, hi = c * ROWS, (c + 1) * ROWS
        if hi <= N:
            xs = logits[lo:hi]
            gc = gvals[lo:hi]
        else:
            xs = np.concatenate(
                [logits[lo:], np.zeros((hi - N, C), np.float32)], axis=0
            )
            gc = np.concatenate([gvals[lo:], np.zeros(hi - N, np.float32)])
        in_maps.append(
            {
                "x": np.ascontiguousarray(xs.reshape(P, SPP * C)),
                "g": np.ascontiguousarray(gc.reshape(P, SPP)),
            }
        )

    trace = bool(int(os.environ.get("ECE_TRACE", "0")))
    res = run_bass_kernel_spmd(
        _get_nc(), in_maps, core_ids=list(range(NCORES)), trace=trace
    )
    LAST_RESULTS = res

    # host epilogue: conf/acc from the per-sample device outputs, then the
    # reference binning in float64 (pads are the positional tail; drop them)
    em = np.concatenate(
        [out["em"].reshape(-1) for out in res.results]
    )[:N]
    s = np.concatenate(
        [out["s"].reshape(-1) for out in res.results]
    )[:N]
    gb = np.concatenate(
        [out["gb"].reshape(-1) for out in res.results]
    )[:N]

    conf = em.astype(np.float64) / s.astype(np.float64)
    acc = (
        gb.view(np.uint16) == em.view(np.uint16)
    ).astype(np.float64)

    NB = 15
    bin_ids = np.clip(np.ceil(conf * NB).astype(np.int64) - 1, 0, NB)
    conf_sum = np.bincount(bin_ids, weights=conf, minlength=NB + 1)
    acc_sum = np.bincount(bin_ids, weights=acc, minlength=NB + 1)
    ece = np.abs(conf_sum - acc_sum).sum() / N
    return np.array([ece], dtype=np.float32)
